# revision 11
# baseline (speedup 1.0000x reference)
"""Trainium2 Bass kernel for nn_Attention_85581518340337.

Restormer-style channel attention:
  x (1,64,16,64,64) -> 1x1x1 conv (64->768) -> grouped 3x3x3 conv (192 groups of 4)
  -> split q,k,v (4 heads x 64 ch) -> L2 normalize over n=t*h*w -> attn = softmax(q@kT * temp)
  -> out = attn@v -> 1x1x1 proj (256->64)

Sharding: spatial over H (64 rows -> 8 cores x 8 rows, halo 1 row each side).

q,k conv: folded (qkv1*dwconv) dense conv computed in fp8 (e4m3, DoubleRow):
staging XS[128, 18t, 2s, 640] holds w-shifted copies (band0: w+0/w+1 via Ko,
band1: w+2/w+3); each (dti,dhi) is one DoubleRow matmul contracting 256
(64ch x 4 w-taps), 9 MMs per t-plane vs 14 bf16 MMs. Weights pre-scaled by a
power of two S (cancels in L2 normalization).

q,k DMA-transposed (bf16) and reduced to per-head Gram matrices on PE; one
133KB AllReduce of Gram partials; softmax + projection folded into per-head
64x64 matrices B_h, split per head-pair: out = G0@x + G1@x where Gp =
concat(B_h o Mfold_v). The v-conv runs in bf16 (xa/xb/xc staging, 14 slots),
col-tiled 2 t-planes per pass (M=64 each), pass 0 (heads 0,1) overlapping the
pair-1 AllReduce latency.
"""

import numpy as np
import ml_dtypes

import concourse.bass as bass
import concourse.mybir as mybir
import concourse.tile as tile
from concourse import bacc
from concourse.bass_utils import run_bass_kernel_spmd

F32 = mybir.dt.float32
BF16 = mybir.dt.bfloat16
F8 = mybir.dt.float8e4

N_CORES = 8
DIM = 64
HEADS = 4
T, H, W = 16, 64, 64
HL = H // N_CORES          # 8 output h-rows per core
HLH = HL + 2               # 10 h-rows incl halo
C3H = DIM * 3 * HEADS      # 768
N_LOC = T * HL * W         # 8192 output positions per core
NT = 512                   # matmul free tile = one t-plane (8*64)

# fp8 staging layout: XS[128, XT, 2, SUB]
XT = T + 2                 # 18 t-planes incl halo
SUB = HLH * W              # 640 = 10 rows x 64 (Ko stride, %16==0)

# bf16 v-conv staging layout (t, h, w) = (18, 10, 68)
PT, PH, PW = T + 2, HLH, W + 4
PLANE = PH * PW            # 680
PFREE = PT * PLANE         # 12240

_CACHE = {}

EXP = mybir.ActivationFunctionType.Exp
LN = mybir.ActivationFunctionType.Ln


def _build(sim=False, stop_after=None):
    nc = bacc.Bacc("TRN2", target_bir_lowering=False, debug=False,
                   num_devices=1 if sim else N_CORES)

    xs8_d = nc.dram_tensor("xs8", [128, XT, 2, SUB], F8, kind="ExternalInput").ap()
    dwt8_d = nc.dram_tensor("dwt8", [128, 4, 9, 2, 128], F8, kind="ExternalInput").ap()
    x_d = nc.dram_tensor("x", [DIM, PT * PH * PW], BF16, kind="ExternalInput").ap()
    dwtv_d = nc.dram_tensor("dwtv", [14, 2, 128, 128], BF16, kind="ExternalInput").ap()
    projt_d = nc.dram_tensor("projt", [128, 2, DIM], F32, kind="ExternalInput").ap()
    temp_d = nc.dram_tensor("temp", [HEADS], F32, kind="ExternalInput").ap()
    eye_d = nc.dram_tensor("eye", [128, 128], F32, kind="ExternalInput").ap()
    out_d = nc.dram_tensor("out", [DIM, T, HL, W], F32, kind="ExternalOutput").ap()

    with tile.TileContext(nc) as tc:
        _emit(nc, tc, xs8_d, dwt8_d, x_d, dwtv_d, projt_d, temp_d, eye_d, out_d,
              sim=sim, stop_after=stop_after)
    nc.compile()
    return nc


def _emit(nc, tc, xs8_d, dwt8_d, x_d, dwtv_d, projt_d, temp_d, eye_d, out_d,
          sim=False, stop_after=None):
    import contextlib
    ctx = contextlib.ExitStack()
    with ctx:
        singles = ctx.enter_context(tc.tile_pool(name="singles", bufs=1))
        dense_p = ctx.enter_context(tc.tile_pool(name="dense", bufs=1))
        ct_p = ctx.enter_context(tc.tile_pool(name="ctp", bufs=3))
        small_p = ctx.enter_context(tc.tile_pool(name="small", bufs=2))
        out_p = ctx.enter_context(tc.tile_pool(name="outp", bufs=2))
        ps_conv = ctx.enter_context(tc.tile_pool(name="ps_conv", bufs=3, space="PSUM"))
        ps_gram = ctx.enter_context(tc.tile_pool(name="ps_gram", bufs=1, space="PSUM"))
        ps_fo = ctx.enter_context(tc.tile_pool(name="ps_fo", bufs=2, space="PSUM"))
        dram = ctx.enter_context(tc.tile_pool(name="dram", bufs=1, space="DRAM"))

        # ---- fp8 conv inputs: weights (macro 0 first), then XS in t-chunks ----
        dwt8_sb = singles.tile([128, 4, 9, 2, 128], F8)
        nc.sync.dma_start(out=dwt8_sb[:, 0], in_=dwt8_d[:, 0])
        xs = singles.tile([128, XT, 2, SUB], F8)
        for j in range(6):
            nc.sync.dma_start(out=xs[:, 3 * j:3 * j + 3], in_=xs8_d[:, 3 * j:3 * j + 3])
        nc.sync.dma_start(out=dwt8_sb[:, 1:4], in_=dwt8_d[:, 1:4])

        # ---- act table preload (ln/exp share one table) ----
        actp = singles.tile([1, 2], F32)
        nc.gpsimd.memset(actp[:], 1.0)
        nc.scalar.activation(out=actp[:, 1:2], in_=actp[:, 0:1], func=LN)

        # ---- small inputs ----
        projt_sb = singles.tile([128, 2, DIM], F32)
        eye_sb = singles.tile([128, 128], F32)
        tsc = singles.tile([128, 2], F32)
        nc.sync.dma_start(out=projt_sb[:], in_=projt_d)
        nc.sync.dma_start(out=eye_sb[:], in_=eye_d)
        for p_ in range(2):
            for hf_ in range(2):
                src_ = bass.AP(tensor=temp_d.tensor, offset=2 * p_ + hf_,
                               ap=[[0, 64], [1, 1]])
                nc.sync.dma_start(out=tsc[hf_ * 64:(hf_ + 1) * 64, p_:p_ + 1], in_=src_)

        # ---- bf16 staging for the v-conv (loaded during conv phase) ----
        # xa: band0 w-shift 0, band1 w-shift +1
        # xb: band0 w-shift 0, band1 h-shift +1
        # xc: band0 w-shift 0, band1 t-shift +1
        xa = singles.tile([128, PT, PH, PW], BF16)
        xb = singles.tile([128, PT, PH, PW], BF16)
        xc = singles.tile([128, PT, PH, PW], BF16)
        xaf = xa[:].rearrange("p t h w -> p (t h w)")
        xbf = xb[:].rearrange("p t h w -> p (t h w)")
        xcf = xc[:].rearrange("p t h w -> p (t h w)")
        # only the unwritten tails of the shifted band-1 loads need zeroing
        nc.gpsimd.memset(xaf[64:128, PFREE - 1:PFREE], 0.0)
        nc.gpsimd.memset(xbf[64:128, PFREE - PW:PFREE], 0.0)
        nc.gpsimd.memset(xcf[64:128, PFREE - PLANE:PFREE], 0.0)
        nc.sync.dma_start(out=xaf[0:64, :], in_=x_d)
        nc.sync.dma_start(out=xaf[64:128, 0:PFREE - 1], in_=x_d[:, 1:])
        nc.sync.dma_start(out=xbf[0:64, :], in_=x_d)
        nc.sync.dma_start(out=xbf[64:128, 0:PFREE - PW], in_=x_d[:, PW:])
        nc.sync.dma_start(out=xcf[0:64, :], in_=x_d)
        nc.sync.dma_start(out=xcf[64:128, 0:PFREE - PLANE], in_=x_d[:, PLANE:])
        dwtv_sb = singles.tile([128, 14, 2, 128], BF16)
        nc.sync.dma_start(out=dwtv_sb[:], in_=dwtv_d.rearrange("s p k m -> k s p m"))

        # dense bf16 buffers for q,k (to transpose for the gram)
        qk_dense = [dense_p.tile([128, N_LOC], BF16, tag=f"qk{m}", name=f"qk{m}")
                    for m in range(4)]

        gq_ps = [None, None]
        arbuf = singles.tile([128, 2, 130], F32)
        ssqk = singles.tile([128, 2, T], F32)

        # macro order: q0, k0, q1, k1 (qkv ch-macros 0,2,1,3); v folded through attn
        macro_order = [0, 2, 1, 3]

        def conv_macro(mac, after_plane=None):
            """fp8 DoubleRow folded conv for one 128-channel macro tile:
            9 (dti,dhi) matmuls per t-plane, each contracting 256 =
            64ch x 4 w-taps (bands w+0/w+2, Ko w+0/w+1)."""
            for t in range(T):
                dtis = [d for d in range(3) if 0 <= t + d - 1 <= T - 1]
                slots = [(dt, dh) for dt in dtis for dh in range(3)]
                ps = ps_conv.tile([128, NT], F32, tag="cps")
                for i, (dt, dh) in enumerate(slots):
                    nc.tensor.matmul(
                        ps[:], dwt8_sb[:, mac, 3 * dt + dh],
                        xs[:, t + dt, :, W * dh:W * dh + NT],
                        start=(i == 0), stop=(i == len(slots) - 1),
                        perf_mode=mybir.MatmulPerfMode.DoubleRow)
                dst = qk_dense[mac][:, t * NT:(t + 1) * NT]
                if t % 2 == 0:
                    nc.vector.tensor_copy(out=dst, in_=ps[:])
                else:
                    nc.scalar.copy(out=dst, in_=ps[:])
                if mac >= 2:
                    # running sum-of-squares of k on DVE (for k norms)
                    scr = small_p.tile([128, NT], F32, tag="ttr")
                    nc.vector.tensor_mul(scr[:], dst, dst)
                    nc.vector.tensor_reduce(out=ssqk[:, mac - 2, t:t + 1], in_=scr[:],
                                            axis=mybir.AxisListType.X,
                                            op=mybir.AluOpType.add)
                if after_plane is not None:
                    after_plane(t)

        NG = N_LOC // 1024     # 8 transpose groups per pair
        ct_tiles = {}

        def gram_transpose(p, g):
            ct = ct_p.tile([128, 8, 256], BF16, tag="ct", name=f"ct{p}_{g}")
            ct_tiles[(p, g)] = ct
            sl = slice(g * 1024, (g + 1) * 1024)
            nc.sync.dma_start(out=ct[:, :, 0:128], in_=qk_dense[p][:, sl],
                              transpose=True)
            nc.sync.dma_start(out=ct[:, :, 128:256], in_=qk_dense[2 + p][:, sl],
                              transpose=True)

        def gram_mms(p, g):
            ct = ct_tiles.pop((p, g))
            for j in range(8):
                jj = g * 8 + j
                nc.tensor.matmul(gq_ps[p][:], ct[:, j, 0:128], ct[:, j, :],
                                 start=(jj == 0), stop=(jj == N_LOC // 128 - 1))

        def gram_interleave(p):
            """after_plane hook: transpose group g after plane 2g+1, gram MMs
            one group behind (transpose has 2 conv planes to complete)."""
            gq_ps[p] = ps_gram.tile([128, 256], F32, tag="gq", name=f"gq{p}")

            def hook(t):
                if t % 2 == 1:
                    g = (t - 1) // 2
                    gram_transpose(p, g)
                    if g > 0:
                        gram_mms(p, g - 1)
            return hook

        def gram_pair(p):
            """non-interleaved fallback: full gram for pair p."""
            gq_ps[p] = ps_gram.tile([128, 256], F32, tag="gq", name=f"gq{p}")
            for g in range(NG):
                gram_transpose(p, g)
                gram_mms(p, g)

        def extract_pair(p):
            """S block + diagonals of pair p into arbuf[:, p, :]."""
            nc.vector.tensor_copy(out=arbuf[:, p, 0:128], in_=gq_ps[p][:, 128:256])
            scr = small_p.tile([128, 128], F32, tag="scr")
            nc.vector.tensor_mul(scr[:], gq_ps[p][:, 0:128], eye_sb[:])
            nc.vector.tensor_reduce(out=arbuf[:, p, 128:129], in_=scr[:],
                                    axis=mybir.AxisListType.X, op=mybir.AluOpType.add)
            nc.vector.tensor_reduce(out=arbuf[:, p, 129:130], in_=ssqk[:, p, :],
                                    axis=mybir.AxisListType.X, op=mybir.AluOpType.add)

        ar_in = [dram.tile([128, 130], F32, name=f"ar_in{p}") for p in range(2)]
        ar_out = [dram.tile([128, 130], F32, name=f"ar_out{p}") for p in range(2)]
        gar = singles.tile([128, 2, 130], F32)

        def launch_ar(p):
            nc.gpsimd.dma_start(out=ar_in[p][:], in_=arbuf[:, p, :])
            if sim:
                nc.gpsimd.dma_start(out=ar_out[p][:], in_=ar_in[p][:])
            else:
                nc.gpsimd.collective_compute(
                    "AllReduce", mybir.AluOpType.add,
                    replica_groups=[list(range(N_CORES))],
                    ins=[ar_in[p].opt()], outs=[ar_out[p].opt()])
            nc.gpsimd.dma_start(out=gar[:, p, :], in_=ar_out[p][:])

        # ---- per-pair: normalization, softmax, B, compose G ----
        rno = singles.tile([128, 2, 2], F32)
        rqs = singles.tile([128, 2], F32)
        rkb = singles.tile([128, 2, 128], F32)
        rkrow = singles.tile([1, 2, 128], F32)
        bt_sb = [singles.tile([128, DIM], BF16, tag=f"bt{p}", name=f"bt{p}")
                 for p in range(2)]
        gv = [singles.tile([128, 14, DIM], BF16, tag=f"gv{p}", name=f"gv{p}")
              for p in range(2)]

        def chain_pair(p):
            """normalize + softmax + B^T + compose G_p (PE ops: transpose, 2 btp,
            14 compose MMs)."""
            # rno = exp(-0.5*ln(diag)) = rsqrt(diag); same act table as Exp
            nc.scalar.activation(out=rno[:, p, :], in_=gar[:, p, 128:130], func=LN)
            nc.scalar.activation(out=rno[:, p, :], in_=rno[:, p, :], func=EXP,
                                 scale=-0.5)
            nc.vector.tensor_mul(rqs[:, p:p + 1], rno[:, p, 0:1], tsc[:, p:p + 1])
            # rk column -> row (PE transpose) -> all partitions (gpsimd bcast)
            tps = ps_gram.tile([128, 128], F32, tag="aux", name=f"tps{p}")
            nc.tensor.transpose(tps[0:1, :], rno[:, p, 1:2], eye_sb[:])
            nc.vector.tensor_copy(out=rkrow[:, p, :], in_=tps[0:1, :])
            nc.gpsimd.partition_broadcast(rkb[:, p, :], rkrow[:, p, :])

            lg = small_p.tile([128, 128], F32, tag="lg")
            nc.vector.tensor_mul(lg[:], gar[:, p, 0:128], rkb[:, p, :])
            nc.vector.tensor_scalar_mul(lg[:], lg[:], rqs[:, p:p + 1])
            btp = ps_gram.tile([128, 128], F32, tag="aux", name=f"btp{p}")
            mx = small_p.tile([128, 1], F32, tag="mx")
            at = small_p.tile([128, 64], F32, tag="at")
            sm = small_p.tile([128, 1], F32, tag="sm")
            for hf in range(2):
                hs = slice(hf * 64, (hf + 1) * 64)
                sub = lg[hs, hf * 64:(hf + 1) * 64]
                nc.vector.tensor_reduce(out=mx[hs], in_=sub, axis=mybir.AxisListType.X,
                                        op=mybir.AluOpType.max, negate=True)
                nc.scalar.activation(out=at[hs], in_=sub, func=EXP,
                                     bias=mx[hs], scale=1.0)
                nc.vector.tensor_reduce(out=sm[hs], in_=at[hs],
                                        axis=mybir.AxisListType.X,
                                        op=mybir.AluOpType.add)
                nc.vector.reciprocal(out=sm[hs], in_=sm[hs])
                nc.vector.tensor_scalar_mul(at[hs], at[hs], sm[hs])
                # B_h^T = attn_h^T @ projT_h  (partitions hf*64.. aligned throughout)
                nc.tensor.matmul(btp[hs, 0:DIM], at[hs], projt_sb[hs, p, :],
                                 start=True, stop=True)
            nc.vector.tensor_copy(out=bt_sb[p][:], in_=btp[:, 0:DIM])
            # compose G_p: per slot GT[(band,c), e] = MfoldT_v,p . B_p
            for slot in range(14):
                gts = ps_gram.tile([128, 128], F32, tag="aux", name=f"gts{p}_{slot}")
                nc.tensor.matmul(gts[:, 0:DIM], dwtv_sb[:, slot, p, :], bt_sb[p][:],
                                 start=True, stop=True)
                nc.vector.tensor_copy(out=gv[p][:, slot, :], in_=gts[:, 0:DIM])

        # ---- conv phase with gram + pair-0 chain interleaved ----
        if stop_after == "inputs":
            nc.gpsimd.dma_start(out=out_d[:, 0], in_=xa[0:64, 0, 0:8, 0:64])
            return
        for mi, mac in enumerate(macro_order):
            hook = None
            if stop_after != "convonly":
                if mi == 2:
                    hook = gram_interleave(0)
                elif mi == 3:
                    hook = gram_interleave(1)
            conv_macro(mac, after_plane=hook)
            if mi == 2 and stop_after != "convonly":
                gram_mms(0, NG - 1)
                extract_pair(0)
                launch_ar(0)
        if stop_after == "convonly":
            return
        chain_pair(0)
        gram_mms(1, NG - 1)
        extract_pair(1)
        launch_ar(1)

        if stop_after in ("conv",):
            return

        # ---- v-conv: out = G0@x + G1@x, col-tiled 2 t-planes per pass ----
        def vslot_rhs(t, slot):
            if slot < 9:
                dti, dhi = slot // 3, slot % 3
                return xa[:, t + dti, dhi:dhi + 8, 1:65]
            if slot < 12:
                return xb[:, t + (slot - 9), 0:8, 3:67]
            if slot == 12:
                return xc[:, t, 2:10, 3:67]
            return xa[:, t + 2, 2:10, 3:67]

        def vslot_valid(t, slot):
            # A-slots: taps (dti, dhi, 0/1); B: (dti, 0/1, 2); C: (0/1, 2, 2);
            # 13: (2,2,2). Skip slots whose taps all read the zero t-halo.
            if slot < 9:
                dti = slot // 3
            elif slot < 12:
                dti = slot - 9
            elif slot == 12:
                return True          # mixes dti 0 and 1; halo is zero anyway
            else:
                dti = 2
            return 0 <= t + dti - 1 <= T - 1 or (slot == 12)

        out0 = dense_p.tile([128, N_LOC], BF16, tag="qk0", name="out0")

        def vconv_pass(p):
            for tp in range(T // 2):
                t0, t1 = 2 * tp, 2 * tp + 1
                fo = ps_fo.tile([128, NT], F32, tag="fo")
                sl0 = [s for s in range(14) if vslot_valid(t0, s)]
                sl1 = [s for s in range(14) if vslot_valid(t1, s)]
                for s in range(14):
                    if s in sl0:
                        nc.tensor.matmul(fo[0:64, :], gv[p][:, s, :], vslot_rhs(t0, s),
                                         start=(s == sl0[0]), stop=(s == sl0[-1]),
                                         tile_position=(0, 0))
                    if s in sl1:
                        nc.tensor.matmul(fo[64:128, :], gv[p][:, s, :], vslot_rhs(t1, s),
                                         start=(s == sl1[0]), stop=(s == sl1[-1]),
                                         tile_position=(0, 64))
                if p == 0:
                    nc.vector.tensor_copy(out=out0[:, tp * NT:(tp + 1) * NT], in_=fo[:])
                else:
                    ot = out_p.tile([128, NT], F32, tag="ot")
                    nc.vector.tensor_add(ot[:], fo[:], out0[:, tp * NT:(tp + 1) * NT])
                    nc.sync.dma_start(
                        out=out_d[:, t0],
                        in_=ot[0:64].rearrange("p (h w) -> p h w", h=HL))
                    nc.sync.dma_start(
                        out=out_d[:, t1],
                        in_=ot[64:128].rearrange("p (h w) -> p h w", h=HL))

        vconv_pass(0)
        chain_pair(1)
        vconv_pass(1)


def _prep_inputs(x, qkv_w, dw_w, proj_w, temperature):
    """Host-side sharding + weight layout."""
    b, c, t, h, w = x.shape
    w1 = qkv_w.reshape(C3H, DIM).astype(np.float64)   # (768, 64)
    dw = dw_w.reshape(C3H, 4, 3, 3, 3).astype(np.float64)
    # folded conv: M[o, c, dti, dhi, dwi] = sum_j dw[o, j, taps] * w1[4*(o//4)+j, c]
    j_idx = (np.arange(C3H) // 4) * 4
    w1g = w1[j_idx[:, None] + np.arange(4)[None, :], :]      # (768, 4, 64)
    mfold = np.einsum("ojtuv,ojc->octuv", dw, w1g)           # (768, 64, 3,3,3)

    # ---- fp8 q,k weights: [c_band(128), mac(4), slot(9), ko(2), o(128)] ----
    mqk = mfold[0:512]                                       # (512, 64, 3,3,3)
    S = 2.0 ** np.floor(np.log2(120.0 / np.abs(mqk).max()))
    dwt8 = np.zeros((128, 4, 9, 2, 128), dtype=np.float32)
    for mac in range(4):
        osl = slice(mac * 128, (mac + 1) * 128)
        for dt in range(3):
            for dh in range(3):
                for bb in range(2):
                    for s in range(2):
                        dwi = 2 * bb + s
                        if dwi > 2:
                            continue
                        dwt8[64 * bb:64 * bb + 64, mac, 3 * dt + dh, s, :] = \
                            (S * mqk[osl, :, dt, dh, dwi]).T
    dwt8 = np.clip(dwt8, -240, 240).astype(ml_dtypes.float8_e4m3fn)

    # ---- fp8 x image: xq8[c, tt, hh, ww] padded (t+-1, h+-1, w: -1..+2) ----
    xq = np.zeros((c, XT, H + 2, W + 3), dtype=np.float32)
    xq[:, 1:T + 1, 1:H + 1, 1:W + 1] = x[0]
    xq8 = xq.astype(ml_dtypes.float8_e4m3fn)

    # ---- v-conv slots (bf16 path), as in the baseline ----
    slots = []
    for dti in range(3):
        for dhi in range(3):
            slots.append(((dti, dhi, 0), (dti, dhi, 1)))     # A-pairs
    for dti in range(3):
        slots.append(((dti, 0, 2), (dti, 1, 2)))             # B-pairs (h-shift band)
    slots.append(((0, 2, 2), (1, 2, 2)))                     # C-pair (t-shift band)
    slots.append(((2, 2, 2), None))                          # single
    # dwtv[s, p, o, 64b + c] = mfold[512 + 128p + o, c, tap(s, b)]
    dwtv = np.zeros((14, 2, 128, 128), dtype=np.float32)
    for si, (tap0, tap1) in enumerate(slots):
        for p in range(2):
            osl = slice(512 + p * 128, 512 + (p + 1) * 128)
            dwtv[si, p, :, 0:64] = mfold[osl, :, tap0[0], tap0[1], tap0[2]]
            if tap1 is not None:
                dwtv[si, p, :, 64:128] = mfold[osl, :, tap1[0], tap1[1], tap1[2]]
    dwtv = dwtv.astype(ml_dtypes.bfloat16)
    pw = proj_w.reshape(DIM, HEADS, DIM)              # (e, h, c)
    # projt[hf*64+c, p, e] = proj_w[e, (2p+hf)*64 + c]
    projt = np.zeros((128, 2, DIM), dtype=np.float32)
    for p in range(2):
        for hf in range(2):
            projt[hf * 64:(hf + 1) * 64, p, :] = pw[:, 2 * p + hf, :].T
    temp = np.asarray(temperature, dtype=np.float32).reshape(HEADS)
    eye = np.eye(128, dtype=np.float32)

    xp = np.zeros((c, t, h + 2, w), dtype=np.float32)
    xp[:, :, 1:h + 1, :] = x[0]
    in_maps = []
    for i in range(N_CORES):
        R0 = i * HL
        # fp8 staging [128, 18, 2, 640]
        xs8 = np.zeros((128, XT, 2, SUB), dtype=ml_dtypes.float8_e4m3fn)
        for bb in range(2):
            for s in range(2):
                sl = xq8[:, :, R0:R0 + HLH, 2 * bb + s:2 * bb + s + W]
                xs8[64 * bb:64 * bb + 64, :, s, :] = sl.reshape(c, XT, SUB)
        # bf16 v-conv image
        xsb = np.zeros((c, PT, PH, PW), dtype=np.float32)
        xsb[:, 1:T + 1, :, 2:W + 2] = xp[:, :, R0:R0 + HLH, :]
        xsb = xsb.reshape(c, PT * PH * PW).astype(ml_dtypes.bfloat16)
        in_maps.append({"xs8": xs8, "dwt8": dwt8, "x": xsb, "dwtv": dwtv,
                        "projt": projt, "temp": temp, "eye": eye})
    return in_maps


def kernel(x, qkv_w, dw_w, proj_w, temperature, _trace=False):
    if "nc" not in _CACHE:
        _CACHE["nc"] = _build()
    nc = _CACHE["nc"]
    in_maps = _prep_inputs(np.asarray(x, np.float32), np.asarray(qkv_w, np.float32),
                           np.asarray(dw_w, np.float32), np.asarray(proj_w, np.float32),
                           np.asarray(temperature, np.float32))
    kw = {}
    if _trace:
        kw = dict(trace=True, stitch_traces=True, trace_cores=list(range(N_CORES)))
    res = run_bass_kernel_spmd(nc, in_maps, core_ids=list(range(N_CORES)), **kw)
    _CACHE["last_res"] = res
    out = np.zeros((1, DIM, T, H, W), dtype=np.float32)
    for i in range(N_CORES):
        out[0, :, :, i * HL:(i + 1) * HL, :] = res.results[i]["out"]
    return out


# revision 41
# speedup vs baseline: 1.4340x; 1.4340x over previous
"""Trainium2 Bass kernel for nn_Attention_85581518340337.

Restormer-style channel attention:
  x (1,64,16,64,64) -> 1x1x1 conv (64->768) -> grouped 3x3x3 conv (192 groups of 4)
  -> split q,k,v (4 heads x 64 ch) -> L2 normalize over n=t*h*w -> attn = softmax(q@kT * temp)
  -> out = attn@v -> 1x1x1 proj (256->64)

Sharding: spatial over H (64 rows -> 8 cores x 8 rows, halo 1 row each side).

q,k conv: folded (qkv1*dwconv) dense conv computed in fp8 (e4m3, DoubleRow):
staging XS[128, 18t, 2s, 640] holds w-shifted copies (band0: w+0/w+1 via Ko,
band1: w+2/w+3); each (dti,dhi) is one DoubleRow matmul contracting 256
(64ch x 4 w-taps), 9 MMs per t-plane vs 14 bf16 MMs. Weights pre-scaled by a
power of two S (cancels in L2 normalization).

q,k DMA-transposed (bf16) and reduced to per-head Gram matrices on PE; one
133KB AllReduce of Gram partials; softmax + projection folded into per-head
64x64 matrices B_h, split per head-pair: out = G0@x + G1@x where Gp =
concat(B_h o Mfold_v). The v-conv runs in bf16 (xa/xb/xc staging, 14 slots),
col-tiled 2 t-planes per pass (M=64 each), pass 0 (heads 0,1) overlapping the
pair-1 AllReduce latency.
"""

import numpy as np
import ml_dtypes

import concourse.bass as bass
import concourse.mybir as mybir
import concourse.tile as tile
from concourse import bacc
from concourse.bass_utils import run_bass_kernel_spmd

F32 = mybir.dt.float32
BF16 = mybir.dt.bfloat16
F8 = mybir.dt.float8e4

N_CORES = 8
DIM = 64
HEADS = 4
T, H, W = 16, 64, 64
HL = H // N_CORES          # 8 output h-rows per core
HLH = HL + 2               # 10 h-rows incl halo
C3H = DIM * 3 * HEADS      # 768
N_LOC = T * HL * W         # 8192 output positions per core
NT = 512                   # matmul free tile = one t-plane (8*64)

# fp8 staging layout: XS[128, XT, 2, SUB]
XT = T + 2                 # 18 t-planes incl halo
SUBV = HLH * W             # 640 = 10 rows x 64 valid data per sub-plane
SUB = 1024                 # padded sub-plane (Ko stride; 1024 hits the fast path)
NVSLOT = 15                # v-conv slots (9 A + 3 B + 3 C')

# bf16 v-conv staging layout (t, h, w) = (18, 10, 68)
PT, PH, PW = T + 2, HLH, W + 4
PLANE = PH * PW            # 680
PFREE = PT * PLANE         # 12240

_CACHE = {}

EXP = mybir.ActivationFunctionType.Exp
LN = mybir.ActivationFunctionType.Ln


def _build(sim=False, stop_after=None):
    nc = bacc.Bacc("TRN2", target_bir_lowering=False, debug=False,
                   num_devices=1 if sim else N_CORES)

    xs8_d = nc.dram_tensor("xs8", [128, XT, 2, SUBV], F8, kind="ExternalInput").ap()
    dwt8_d = nc.dram_tensor("dwt8", [128, 4, 9, 2, 128], F8, kind="ExternalInput").ap()
    x_d = nc.dram_tensor("x", [DIM, PT * PH * PW], BF16, kind="ExternalInput").ap()
    dwtv_d = nc.dram_tensor("dwtv", [NVSLOT, 2, 128, 128], BF16,
                            kind="ExternalInput").ap()
    projt_d = nc.dram_tensor("projt", [128, 2, DIM], F32, kind="ExternalInput").ap()
    temp_d = nc.dram_tensor("temp", [HEADS], F32, kind="ExternalInput").ap()
    eye_d = nc.dram_tensor("eye", [128, 128], F32, kind="ExternalInput").ap()
    out_d = nc.dram_tensor("out", [DIM, T, HL, W], F32, kind="ExternalOutput").ap()

    with tile.TileContext(nc) as tc:
        _emit(nc, tc, xs8_d, dwt8_d, x_d, dwtv_d, projt_d, temp_d, eye_d, out_d,
              sim=sim, stop_after=stop_after)
    nc.compile()
    return nc


def _emit(nc, tc, xs8_d, dwt8_d, x_d, dwtv_d, projt_d, temp_d, eye_d, out_d,
          sim=False, stop_after=None):
    import contextlib
    ctx = contextlib.ExitStack()
    with ctx:
        singles = ctx.enter_context(tc.tile_pool(name="singles", bufs=1))
        dense_p = ctx.enter_context(tc.tile_pool(name="dense", bufs=1))
        ct_p = ctx.enter_context(tc.tile_pool(name="ctp", bufs=4))
        small_p = ctx.enter_context(tc.tile_pool(name="small", bufs=2))
        out_p = ctx.enter_context(tc.tile_pool(name="outp", bufs=2))
        ps_conv = ctx.enter_context(tc.tile_pool(name="ps_conv", bufs=4, space="PSUM"))
        ps_gram = ctx.enter_context(tc.tile_pool(name="ps_gram", bufs=1, space="PSUM"))
        ps_fo = ctx.enter_context(tc.tile_pool(name="ps_fo", bufs=2, space="PSUM"))
        dram = ctx.enter_context(tc.tile_pool(name="dram", bufs=1, space="DRAM"))

        # ---- fp8 conv inputs: weights (macro 0 first), then XS in t-chunks ----
        dwt8_sb = singles.tile([128, 4, 9, 2, 128], F8)
        nc.sync.dma_start(out=dwt8_sb[:, 0], in_=dwt8_d[:, 0])
        # pad region [SUBV:SUB] of each sub-plane is never read -> load 640 only
        xs = singles.tile([128, XT, 2, SUB], F8)
        for j in range(6):
            nc.sync.dma_start(out=xs[:, 3 * j:3 * j + 3, :, 0:SUBV],
                              in_=xs8_d[:, 3 * j:3 * j + 3])
        nc.sync.dma_start(out=dwt8_sb[:, 1:4], in_=dwt8_d[:, 1:4])

        # ---- act table preload (ln/exp share one table) ----
        actp = singles.tile([1, 2], F32)
        nc.gpsimd.memset(actp[:], 1.0)
        nc.scalar.activation(out=actp[:, 1:2], in_=actp[:, 0:1], func=LN)

        # ---- small inputs ----
        projt_sb = singles.tile([128, 2, DIM], F32)
        eye_sb = singles.tile([128, 128], F32)
        tsc = singles.tile([128, 2], F32)
        nc.sync.dma_start(out=projt_sb[:], in_=projt_d)
        nc.sync.dma_start(out=eye_sb[:], in_=eye_d)
        for p_ in range(2):
            for hf_ in range(2):
                src_ = bass.AP(tensor=temp_d.tensor, offset=2 * p_ + hf_,
                               ap=[[0, 64], [1, 1]])
                nc.sync.dma_start(out=tsc[hf_ * 64:(hf_ + 1) * 64, p_:p_ + 1], in_=src_)

        # ---- bf16 staging for the v-conv (loaded during conv phase) ----
        # xa: band0 w-shift 0, band1 w-shift +1
        # xb: band0 w-shift 0, band1 h-shift +1
        xa = singles.tile([128, PT, PH, PW], BF16)
        xb = singles.tile([128, PT, PH, PW], BF16)
        xaf = xa[:].rearrange("p t h w -> p (t h w)")
        xbf = xb[:].rearrange("p t h w -> p (t h w)")
        # only the unwritten tails of the shifted band-1 loads need zeroing
        nc.gpsimd.memset(xaf[64:128, PFREE - 1:PFREE], 0.0)
        nc.gpsimd.memset(xbf[64:128, PFREE - PW:PFREE], 0.0)
        nc.sync.dma_start(out=xaf[0:64, :], in_=x_d)
        nc.sync.dma_start(out=xaf[64:128, 0:PFREE - 1], in_=x_d[:, 1:])
        nc.sync.dma_start(out=xbf[0:64, :], in_=x_d)
        nc.sync.dma_start(out=xbf[64:128, 0:PFREE - PW], in_=x_d[:, PW:])
        dwtv_sb = singles.tile([128, NVSLOT, 2, 128], BF16)
        nc.sync.dma_start(out=dwtv_sb[:], in_=dwtv_d.rearrange("s p k m -> k s p m"))

        # dense bf16 buffers for q,k (to transpose for the gram)
        qk_dense = [dense_p.tile([128, N_LOC], BF16, tag=f"qk{m}", name=f"qk{m}")
                    for m in range(4)]

        gq_ps = [None, None]
        arbuf = singles.tile([128, 2, 130], F32)
        ssqk = singles.tile([128, 2, T], F32)

        # macro order: q0, k0, q1, k1 (qkv ch-macros 0,2,1,3); v folded through attn
        macro_order = [0, 2, 1, 3]

        def conv_macro(mac, after_plane=None):
            """fp8 DoubleRow folded conv for one 128-channel macro tile:
            9 (dti,dhi) matmuls per t-plane, each contracting 256 =
            64ch x 4 w-taps (bands w+0/w+2, Ko w+0/w+1 at stride SUB)."""
            for t in range(T):
                dtis = [d for d in range(3) if 0 <= t + d - 1 <= T - 1]
                slots = [(dt, dh) for dt in dtis for dh in range(3)]
                ps = ps_conv.tile([128, NT], F32, tag="cps")
                for i, (dt, dh) in enumerate(slots):
                    nc.tensor.matmul(
                        ps[:], dwt8_sb[:, mac, 3 * dt + dh],
                        xs[:, t + dt, :, W * dh:W * dh + NT],
                        start=(i == 0), stop=(i == len(slots) - 1),
                        perf_mode=mybir.MatmulPerfMode.DoubleRow)
                dst = qk_dense[mac][:, t * NT:(t + 1) * NT]
                if t % 2 == 0:
                    nc.vector.tensor_copy(out=dst, in_=ps[:])
                else:
                    nc.scalar.copy(out=dst, in_=ps[:])
                if mac >= 2:
                    # running sum-of-squares of k on DVE (for k norms)
                    scr = small_p.tile([128, NT], F32, tag="ttr")
                    nc.vector.tensor_mul(scr[:], dst, dst)
                    nc.vector.tensor_reduce(out=ssqk[:, mac - 2, t:t + 1], in_=scr[:],
                                            axis=mybir.AxisListType.X,
                                            op=mybir.AluOpType.add)
                if after_plane is not None:
                    after_plane(t)

        NG = N_LOC // 1024     # 8 transpose groups per pair
        ct_tiles = {}

        def gram_transpose(p, g):
            ct = ct_p.tile([128, 8, 256], BF16, tag="ct", name=f"ct{p}_{g}")
            ct_tiles[(p, g)] = ct
            sl = slice(g * 1024, (g + 1) * 1024)
            nc.sync.dma_start(out=ct[:, :, 0:128], in_=qk_dense[p][:, sl],
                              transpose=True)
            nc.sync.dma_start(out=ct[:, :, 128:256], in_=qk_dense[2 + p][:, sl],
                              transpose=True)

        def gram_mms(p, g):
            ct = ct_tiles.pop((p, g))
            for j in range(8):
                jj = g * 8 + j
                nc.tensor.matmul(gq_ps[p][:], ct[:, j, 0:128], ct[:, j, :],
                                 start=(jj == 0), stop=(jj == N_LOC // 128 - 1))

        def gram_interleave(p):
            """after_plane hook: transpose group g after plane 2g+1, gram MMs
            two groups behind (transpose has a full conv chunk to complete)."""
            gq_ps[p] = ps_gram.tile([128, 256], F32, tag="gq", name=f"gq{p}")

            def hook(t):
                if t % 2 == 1:
                    g = (t - 1) // 2
                    gram_transpose(p, g)
                    if g > 1:
                        gram_mms(p, g - 2)
            return hook

        def gram_pair(p):
            """non-interleaved fallback: full gram for pair p."""
            gq_ps[p] = ps_gram.tile([128, 256], F32, tag="gq", name=f"gq{p}")
            for g in range(NG):
                gram_transpose(p, g)
                gram_mms(p, g)

        def extract_pair(p):
            """S block + diagonals of pair p into arbuf[:, p, :]."""
            nc.vector.tensor_copy(out=arbuf[:, p, 0:128], in_=gq_ps[p][:, 128:256])
            scr = small_p.tile([128, 128], F32, tag="scr")
            nc.vector.tensor_mul(scr[:], gq_ps[p][:, 0:128], eye_sb[:])
            nc.vector.tensor_reduce(out=arbuf[:, p, 128:129], in_=scr[:],
                                    axis=mybir.AxisListType.X, op=mybir.AluOpType.add)
            nc.vector.tensor_reduce(out=arbuf[:, p, 129:130], in_=ssqk[:, p, :],
                                    axis=mybir.AxisListType.X, op=mybir.AluOpType.add)

        ar_in = [dram.tile([128, 130], F32, name=f"ar_in{p}") for p in range(2)]
        ar_out = [dram.tile([128, 130], F32, name=f"ar_out{p}") for p in range(2)]
        gar = singles.tile([128, 2, 130], F32)

        def launch_ar(p):
            nc.gpsimd.dma_start(out=ar_in[p][:], in_=arbuf[:, p, :])
            if sim:
                nc.gpsimd.dma_start(out=ar_out[p][:], in_=ar_in[p][:])
            else:
                nc.gpsimd.collective_compute(
                    "AllReduce", mybir.AluOpType.add,
                    replica_groups=[list(range(N_CORES))],
                    ins=[ar_in[p].opt()], outs=[ar_out[p].opt()])
            nc.gpsimd.dma_start(out=gar[:, p, :], in_=ar_out[p][:])

        # ---- per-pair: normalization, softmax, B, compose G ----
        rno = singles.tile([128, 2, 2], F32)
        rqs = singles.tile([128, 2], F32)
        rkb = singles.tile([128, 2, 128], F32)
        rkrow = singles.tile([1, 2, 128], F32)
        bt_sb = [singles.tile([128, DIM], BF16, tag=f"bt{p}", name=f"bt{p}")
                 for p in range(2)]
        # G = G0 + G1 (out = G0@x + G1@x = (G0+G1)@x): pair 0 stages to gv0,
        # pair 1's compose evacuation adds into gsum
        gv0 = singles.tile([128, NVSLOT, DIM], F32)
        gsum = singles.tile([128, NVSLOT, DIM], BF16)

        def chain_pair(p):
            """normalize + softmax + B^T + compose G_p (PE ops: transpose, 2 btp,
            14 compose MMs)."""
            # rno = exp(-0.5*ln(diag)) = rsqrt(diag); same act table as Exp
            nc.scalar.activation(out=rno[:, p, :], in_=gar[:, p, 128:130], func=LN)
            nc.scalar.activation(out=rno[:, p, :], in_=rno[:, p, :], func=EXP,
                                 scale=-0.5)
            nc.vector.tensor_mul(rqs[:, p:p + 1], rno[:, p, 0:1], tsc[:, p:p + 1])
            # rk column -> row (PE transpose) -> all partitions (gpsimd bcast)
            tps = ps_gram.tile([128, 128], F32, tag="aux", name=f"tps{p}")
            nc.tensor.transpose(tps[0:1, :], rno[:, p, 1:2], eye_sb[:])
            nc.vector.tensor_copy(out=rkrow[:, p, :], in_=tps[0:1, :])
            nc.gpsimd.partition_broadcast(rkb[:, p, :], rkrow[:, p, :])

            lg = small_p.tile([128, 128], F32, tag="lg")
            nc.vector.tensor_mul(lg[:], gar[:, p, 0:128], rkb[:, p, :])
            nc.vector.tensor_scalar_mul(lg[:], lg[:], rqs[:, p:p + 1])
            btp = ps_gram.tile([128, 128], F32, tag="aux", name=f"btp{p}")
            mx = small_p.tile([128, 1], F32, tag="mx")
            at = small_p.tile([128, 64], F32, tag="at")
            sm = small_p.tile([128, 1], F32, tag="sm")
            for hf in range(2):
                hs = slice(hf * 64, (hf + 1) * 64)
                sub = lg[hs, hf * 64:(hf + 1) * 64]
                nc.vector.tensor_reduce(out=mx[hs], in_=sub, axis=mybir.AxisListType.X,
                                        op=mybir.AluOpType.max, negate=True)
                nc.scalar.activation(out=at[hs], in_=sub, func=EXP,
                                     bias=mx[hs], scale=1.0)
                nc.vector.tensor_reduce(out=sm[hs], in_=at[hs],
                                        axis=mybir.AxisListType.X,
                                        op=mybir.AluOpType.add)
                nc.vector.reciprocal(out=sm[hs], in_=sm[hs])
                nc.vector.tensor_scalar_mul(at[hs], at[hs], sm[hs])
                # B_h^T = attn_h^T @ projT_h  (partitions hf*64.. aligned throughout)
                nc.tensor.matmul(btp[hs, 0:DIM], at[hs], projt_sb[hs, p, :],
                                 start=True, stop=True)
            nc.vector.tensor_copy(out=bt_sb[p][:], in_=btp[:, 0:DIM])
            # compose G_p: per slot GT[(band,c), e] = MfoldT_v,p . B_p
            for slot in range(NVSLOT):
                gts = ps_gram.tile([128, 128], F32, tag="aux", name=f"gts{p}_{slot}")
                nc.tensor.matmul(gts[:, 0:DIM], dwtv_sb[:, slot, p, :], bt_sb[p][:],
                                 start=True, stop=True)
                if p == 0:
                    nc.vector.tensor_copy(out=gv0[:, slot, :], in_=gts[:, 0:DIM])
                else:
                    nc.vector.tensor_add(gsum[:, slot, :], gts[:, 0:DIM],
                                         gv0[:, slot, :])

        # ---- conv phase with gram + pair-0 chain interleaved ----
        if stop_after == "inputs":
            nc.gpsimd.dma_start(out=out_d[:, 0], in_=xa[0:64, 0, 0:8, 0:64])
            return
        for mi, mac in enumerate(macro_order):
            hook = None
            if stop_after != "convonly":
                if mi == 2:
                    hook = gram_interleave(0)
                elif mi == 3:
                    hook = gram_interleave(1)
            conv_macro(mac, after_plane=hook)
            if mi == 2 and stop_after != "convonly":
                gram_mms(0, NG - 2)
                gram_mms(0, NG - 1)
                extract_pair(0)
                launch_ar(0)
        if stop_after == "convonly":
            return
        gram_mms(1, NG - 2)
        chain_pair(0)
        gram_mms(1, NG - 1)
        extract_pair(1)
        launch_ar(1)

        if stop_after in ("conv",):
            return

        # ---- v-conv: out = G0@x + G1@x, col-tiled 2 t-planes per pass ----
        def vslot_rhs(t, slot):
            if slot < 9:
                # A-slots: taps (dti, dhi, 0) + (dti, dhi, 1)
                dti, dhi = slot // 3, slot % 3
                return xa[:, t + dti, dhi:dhi + 8, 1:65]
            if slot < 12:
                # B-slots: taps (dti, 0, 2) + (dti, 1, 2)
                return xb[:, t + (slot - 9), 0:8, 3:67]
            # C'-slots: tap (dti, 2, 2) via xb band0, band1 weight 0
            return xb[:, t + (slot - 12), 2:10, 3:67]

        def vslot_valid(t, slot):
            dti = slot // 3 if slot < 9 else (slot - 9 if slot < 12 else slot - 12)
            return 0 <= t + dti - 1 <= T - 1

        def vconv():
            """out plane = (G0+G1) @ x, 15 slots per t-plane, M=64."""
            for t in range(T):
                fo = ps_fo.tile([64, NT], F32, tag="fo")
                sl = [s for s in range(NVSLOT) if vslot_valid(t, s)]
                for i, s in enumerate(sl):
                    nc.tensor.matmul(fo[:], gsum[:, s, :], vslot_rhs(t, s),
                                     start=(i == 0), stop=(i == len(sl) - 1))
                ot = out_p.tile([64, NT], F32, tag="ot")
                if t % 2 == 0:
                    nc.vector.tensor_copy(out=ot[:], in_=fo[:])
                else:
                    nc.scalar.copy(out=ot[:], in_=fo[:])
                nc.sync.dma_start(out=out_d[:, t],
                                  in_=ot[:].rearrange("p (h w) -> p h w", h=HL))

        chain_pair(1)
        vconv()


def _prep_inputs(x, qkv_w, dw_w, proj_w, temperature):
    """Host-side sharding + weight layout."""
    b, c, t, h, w = x.shape
    w1 = qkv_w.reshape(C3H, DIM).astype(np.float64)   # (768, 64)
    dw = dw_w.reshape(C3H, 4, 3, 3, 3).astype(np.float64)
    # folded conv: M[o, c, dti, dhi, dwi] = sum_j dw[o, j, taps] * w1[4*(o//4)+j, c]
    j_idx = (np.arange(C3H) // 4) * 4
    w1g = w1[j_idx[:, None] + np.arange(4)[None, :], :]      # (768, 4, 64)
    mfold = np.einsum("ojtuv,ojc->octuv", dw, w1g)           # (768, 64, 3,3,3)

    # ---- fp8 q,k weights: [c_band(128), mac(4), slot(9), ko(2), o(128)] ----
    mqk = mfold[0:512]                                       # (512, 64, 3,3,3)
    S = 2.0 ** np.floor(np.log2(120.0 / np.abs(mqk).max()))
    dwt8 = np.zeros((128, 4, 9, 2, 128), dtype=np.float32)
    for mac in range(4):
        osl = slice(mac * 128, (mac + 1) * 128)
        for dt in range(3):
            for dh in range(3):
                for bb in range(2):
                    for s in range(2):
                        dwi = 2 * bb + s
                        if dwi > 2:
                            continue
                        dwt8[64 * bb:64 * bb + 64, mac, 3 * dt + dh, s, :] = \
                            (S * mqk[osl, :, dt, dh, dwi]).T
    dwt8 = np.clip(dwt8, -240, 240).astype(ml_dtypes.float8_e4m3fn)

    # ---- fp8 x image: xq8[c, tt, hh, ww] padded (t+-1, h+-1, w: -1..+2) ----
    xq = np.zeros((c, XT, H + 2, W + 3), dtype=np.float32)
    xq[:, 1:T + 1, 1:H + 1, 1:W + 1] = x[0]
    xq8 = xq.astype(ml_dtypes.float8_e4m3fn)

    # ---- v-conv slots (bf16 path) ----
    slots = []
    for dti in range(3):
        for dhi in range(3):
            slots.append(((dti, dhi, 0), (dti, dhi, 1)))     # A-pairs
    for dti in range(3):
        slots.append(((dti, 0, 2), (dti, 1, 2)))             # B-pairs (h-shift band)
    for dti in range(3):
        slots.append(((dti, 2, 2), None))                    # C'-singles
    # dwtv[s, p, o, 64b + c] = mfold[512 + 128p + o, c, tap(s, b)]
    dwtv = np.zeros((NVSLOT, 2, 128, 128), dtype=np.float32)
    for si, (tap0, tap1) in enumerate(slots):
        for p in range(2):
            osl = slice(512 + p * 128, 512 + (p + 1) * 128)
            dwtv[si, p, :, 0:64] = mfold[osl, :, tap0[0], tap0[1], tap0[2]]
            if tap1 is not None:
                dwtv[si, p, :, 64:128] = mfold[osl, :, tap1[0], tap1[1], tap1[2]]
    dwtv = dwtv.astype(ml_dtypes.bfloat16)
    pw = proj_w.reshape(DIM, HEADS, DIM)              # (e, h, c)
    # projt[hf*64+c, p, e] = proj_w[e, (2p+hf)*64 + c]
    projt = np.zeros((128, 2, DIM), dtype=np.float32)
    for p in range(2):
        for hf in range(2):
            projt[hf * 64:(hf + 1) * 64, p, :] = pw[:, 2 * p + hf, :].T
    temp = np.asarray(temperature, dtype=np.float32).reshape(HEADS)
    eye = np.eye(128, dtype=np.float32)

    xp = np.zeros((c, t, h + 2, w), dtype=np.float32)
    xp[:, :, 1:h + 1, :] = x[0]
    in_maps = []
    for i in range(N_CORES):
        R0 = i * HL
        # fp8 staging [128, 18, 2, 640] (device pads sub-planes to stride 1024)
        xs8 = np.zeros((128, XT, 2, SUBV), dtype=ml_dtypes.float8_e4m3fn)
        for bb in range(2):
            for s in range(2):
                sl = xq8[:, :, R0:R0 + HLH, 2 * bb + s:2 * bb + s + W]
                xs8[64 * bb:64 * bb + 64, :, s, :] = sl.reshape(c, XT, SUBV)
        # bf16 v-conv image
        xsb = np.zeros((c, PT, PH, PW), dtype=np.float32)
        xsb[:, 1:T + 1, :, 2:W + 2] = xp[:, :, R0:R0 + HLH, :]
        xsb = xsb.reshape(c, PT * PH * PW).astype(ml_dtypes.bfloat16)
        in_maps.append({"xs8": xs8, "dwt8": dwt8, "x": xsb, "dwtv": dwtv,
                        "projt": projt, "temp": temp, "eye": eye})
    return in_maps


def kernel(x, qkv_w, dw_w, proj_w, temperature, _trace=False):
    if "nc" not in _CACHE:
        _CACHE["nc"] = _build()
    nc = _CACHE["nc"]
    in_maps = _prep_inputs(np.asarray(x, np.float32), np.asarray(qkv_w, np.float32),
                           np.asarray(dw_w, np.float32), np.asarray(proj_w, np.float32),
                           np.asarray(temperature, np.float32))
    kw = {}
    if _trace:
        kw = dict(trace=True, stitch_traces=True, trace_cores=list(range(N_CORES)))
    res = run_bass_kernel_spmd(nc, in_maps, core_ids=list(range(N_CORES)), **kw)
    _CACHE["last_res"] = res
    out = np.zeros((1, DIM, T, H, W), dtype=np.float32)
    for i in range(N_CORES):
        out[0, :, :, i * HL:(i + 1) * HL, :] = res.results[i]["out"]
    return out


# revision 54
# speedup vs baseline: 1.5510x; 1.0816x over previous
"""Trainium2 Bass kernel for nn_Attention_85581518340337.

Restormer-style channel attention:
  x (1,64,16,64,64) -> 1x1x1 conv (64->768) -> grouped 3x3x3 conv (192 groups of 4)
  -> split q,k,v (4 heads x 64 ch) -> L2 normalize over n=t*h*w -> attn = softmax(q@kT * temp)
  -> out = attn@v -> 1x1x1 proj (256->64)

Sharding: spatial over H (64 rows -> 8 cores x 8 rows, halo 1 row each side).

q,k conv: folded (qkv1*dwconv) dense conv computed in fp8 (e4m3, DoubleRow):
staging XS[128, 18t, 2s, 640] holds w-shifted copies (band0: w+0/w+1 via Ko,
band1: w+2/w+3); each (dti,dhi) is one DoubleRow matmul contracting 256
(64ch x 4 w-taps), 9 MMs per t-plane vs 14 bf16 MMs. Weights pre-scaled by a
power of two S (cancels in L2 normalization).

q,k DMA-transposed (bf16) and reduced to per-head cross Gram blocks (q x k,
N=128) on PE, interleaved per-position-group into the conv planes; q,k norms
via DVE sum-of-squares during evacuation; two 66KB AllReduces of Gram
partials; softmax + projection folded into per-head 64x64 matrices B_h;
rsqrt = exp(-0.5*ln(x)) keeps the ACT engine on one table. The v path is
out = (G0+G1)@x where Gp = concat(B_h o Mfold_v): a single bf16 conv
(xa/xb staging, 15 slots, M=64) with two t-planes running concurrently in
separate PE column groups (tile_position col-tiling).
"""

import numpy as np
import ml_dtypes

import concourse.bass as bass
import concourse.mybir as mybir
import concourse.tile as tile
from concourse import bacc
from concourse.bass_utils import run_bass_kernel_spmd

F32 = mybir.dt.float32
BF16 = mybir.dt.bfloat16
F8 = mybir.dt.float8e4

N_CORES = 8
DIM = 64
HEADS = 4
T, H, W = 16, 64, 64
HL = H // N_CORES          # 8 output h-rows per core
HLH = HL + 2               # 10 h-rows incl halo
C3H = DIM * 3 * HEADS      # 768
N_LOC = T * HL * W         # 8192 output positions per core
NT = 512                   # matmul free tile = one t-plane (8*64)

# fp8 staging layout: XS[128, XT, 2, SUB]
XT = T + 2                 # 18 t-planes incl halo
SUBV = HLH * W             # 640 = 10 rows x 64 valid data per sub-plane
SUB = 1024                 # padded sub-plane (Ko stride; 1024 hits the fast path)
NVSLOT = 15                # v-conv slots (9 A + 3 B + 3 C')

# bf16 v-conv staging layout (t, h, w) = (18, 10, 68)
PT, PH, PW = T + 2, HLH, W + 4
PLANE = PH * PW            # 680
PFREE = PT * PLANE         # 12240

_CACHE = {}

EXP = mybir.ActivationFunctionType.Exp
LN = mybir.ActivationFunctionType.Ln


def _build(sim=False, stop_after=None):
    nc = bacc.Bacc("TRN2", target_bir_lowering=False, debug=False,
                   num_devices=1 if sim else N_CORES)

    xs8_d = nc.dram_tensor("xs8", [128, XT, 2, SUBV], F8, kind="ExternalInput").ap()
    dwt8_d = nc.dram_tensor("dwt8", [128, 4, 9, 2, 128], F8, kind="ExternalInput").ap()
    x_d = nc.dram_tensor("x", [DIM, PT * PH * PW], BF16, kind="ExternalInput").ap()
    dwtv_d = nc.dram_tensor("dwtv", [NVSLOT, 2, 128, 128], BF16,
                            kind="ExternalInput").ap()
    projt_d = nc.dram_tensor("projt", [128, 2, DIM], F32, kind="ExternalInput").ap()
    temp_d = nc.dram_tensor("temp", [HEADS], F32, kind="ExternalInput").ap()
    eye_d = nc.dram_tensor("eye", [128, 128], F32, kind="ExternalInput").ap()
    out_d = nc.dram_tensor("out", [DIM, T, HL, W], F32, kind="ExternalOutput").ap()

    with tile.TileContext(nc) as tc:
        _emit(nc, tc, xs8_d, dwt8_d, x_d, dwtv_d, projt_d, temp_d, eye_d, out_d,
              sim=sim, stop_after=stop_after)
    nc.compile()
    return nc


def _emit(nc, tc, xs8_d, dwt8_d, x_d, dwtv_d, projt_d, temp_d, eye_d, out_d,
          sim=False, stop_after=None):
    import contextlib
    ctx = contextlib.ExitStack()
    with ctx:
        singles = ctx.enter_context(tc.tile_pool(name="singles", bufs=1))
        dense_p = ctx.enter_context(tc.tile_pool(name="dense", bufs=1))
        ct_p = ctx.enter_context(tc.tile_pool(name="ctp", bufs=4))
        small_p = ctx.enter_context(tc.tile_pool(name="small", bufs=2))
        out_p = ctx.enter_context(tc.tile_pool(name="outp", bufs=2))
        ps_conv = ctx.enter_context(tc.tile_pool(name="ps_conv", bufs=4, space="PSUM"))
        ps_gram = ctx.enter_context(tc.tile_pool(name="ps_gram", bufs=1, space="PSUM"))
        ps_fo = ctx.enter_context(tc.tile_pool(name="ps_fo", bufs=2, space="PSUM"))
        dram = ctx.enter_context(tc.tile_pool(name="dram", bufs=1, space="DRAM"))

        # ---- fp8 conv inputs: weights (macro 0 first), then XS in t-chunks ----
        dwt8_sb = singles.tile([128, 4, 9, 2, 128], F8)
        nc.sync.dma_start(out=dwt8_sb[:, 0], in_=dwt8_d[:, 0])
        # pad region [SUBV:SUB] of each sub-plane is never read -> load 640 only
        xs = singles.tile([128, XT, 2, SUB], F8)
        for j in range(6):
            nc.sync.dma_start(out=xs[:, 3 * j:3 * j + 3, :, 0:SUBV],
                              in_=xs8_d[:, 3 * j:3 * j + 3])
        nc.sync.dma_start(out=dwt8_sb[:, 1:4], in_=dwt8_d[:, 1:4])

        # ---- act table preload (ln/exp share one table) ----
        actp = singles.tile([1, 2], F32)
        nc.gpsimd.memset(actp[:], 1.0)
        nc.scalar.activation(out=actp[:, 1:2], in_=actp[:, 0:1], func=LN)

        # ---- small inputs ----
        projt_sb = singles.tile([128, 2, DIM], F32)
        eye_sb = singles.tile([128, 128], F32)
        tsc = singles.tile([128, 2], F32)
        nc.sync.dma_start(out=projt_sb[:], in_=projt_d)
        nc.sync.dma_start(out=eye_sb[:], in_=eye_d)
        for p_ in range(2):
            for hf_ in range(2):
                src_ = bass.AP(tensor=temp_d.tensor, offset=2 * p_ + hf_,
                               ap=[[0, 64], [1, 1]])
                nc.sync.dma_start(out=tsc[hf_ * 64:(hf_ + 1) * 64, p_:p_ + 1], in_=src_)

        # ---- bf16 staging for the v-conv (loaded during conv phase) ----
        # xa: band0 w-shift 0, band1 w-shift +1
        # xb: band0 w-shift 0, band1 h-shift +1
        xa = singles.tile([128, PT, PH, PW], BF16)
        xb = singles.tile([128, PT, PH, PW], BF16)
        xaf = xa[:].rearrange("p t h w -> p (t h w)")
        xbf = xb[:].rearrange("p t h w -> p (t h w)")
        # only the unwritten tails of the shifted band-1 loads need zeroing
        nc.gpsimd.memset(xaf[64:128, PFREE - 1:PFREE], 0.0)
        nc.gpsimd.memset(xbf[64:128, PFREE - PW:PFREE], 0.0)
        nc.sync.dma_start(out=xaf[0:64, :], in_=x_d)
        nc.sync.dma_start(out=xaf[64:128, 0:PFREE - 1], in_=x_d[:, 1:])
        nc.sync.dma_start(out=xbf[0:64, :], in_=x_d)
        nc.sync.dma_start(out=xbf[64:128, 0:PFREE - PW], in_=x_d[:, PW:])
        dwtv_sb = singles.tile([128, NVSLOT, 2, 128], BF16)
        nc.sync.dma_start(out=dwtv_sb[:], in_=dwtv_d.rearrange("s p k m -> k s p m"))

        # dense bf16 buffers for q,k (to transpose for the gram)
        qk_dense = [dense_p.tile([128, N_LOC], BF16, tag=f"qk{m}", name=f"qk{m}")
                    for m in range(4)]

        gq_ps = [None, None]
        arbuf = singles.tile([128, 2, 130], F32)
        ssqk = singles.tile([128, 4, T], F32)

        # macro order: q0, k0, q1, k1 (qkv ch-macros 0,2,1,3); v folded through attn
        macro_order = [0, 2, 1, 3]

        def conv_macro(mac, after_plane=None):
            """fp8 DoubleRow folded conv for one 128-channel macro tile:
            9 (dti,dhi) matmuls per t-plane, each contracting 256 =
            64ch x 4 w-taps (bands w+0/w+2, Ko w+0/w+1 at stride SUB)."""
            for t in range(T):
                dtis = [d for d in range(3) if 0 <= t + d - 1 <= T - 1]
                slots = [(dt, dh) for dt in dtis for dh in range(3)]
                ps = ps_conv.tile([128, NT], F32, tag="cps")
                for i, (dt, dh) in enumerate(slots):
                    nc.tensor.matmul(
                        ps[:], dwt8_sb[:, mac, 3 * dt + dh],
                        xs[:, t + dt, :, W * dh:W * dh + NT],
                        start=(i == 0), stop=(i == len(slots) - 1),
                        perf_mode=mybir.MatmulPerfMode.DoubleRow)
                dst = qk_dense[mac][:, t * NT:(t + 1) * NT]
                if t % 2 == 0:
                    nc.vector.tensor_copy(out=dst, in_=ps[:])
                else:
                    nc.scalar.copy(out=dst, in_=ps[:])
                # running sum-of-squares on DVE (q and k norms)
                scr = small_p.tile([128, NT], F32, tag="ttr")
                nc.vector.tensor_mul(scr[:], dst, dst)
                nc.vector.tensor_reduce(out=ssqk[:, mac, t:t + 1], in_=scr[:],
                                        axis=mybir.AxisListType.X,
                                        op=mybir.AluOpType.add)
                if after_plane is not None:
                    after_plane(t)

        NG = N_LOC // 1024     # 8 transpose groups per pair
        ct_tiles = {}

        def gram_transpose(p, g):
            ct = ct_p.tile([128, 8, 256], BF16, tag="ct", name=f"ct{p}_{g}")
            ct_tiles[(p, g)] = ct
            sl = slice(g * 1024, (g + 1) * 1024)
            nc.sync.dma_start(out=ct[:, :, 0:128], in_=qk_dense[p][:, sl],
                              transpose=True)
            nc.sync.dma_start(out=ct[:, :, 128:256], in_=qk_dense[2 + p][:, sl],
                              transpose=True)

        def gram_mms(p, g):
            ct = ct_tiles.pop((p, g))
            for j in range(8):
                jj = g * 8 + j
                nc.tensor.matmul(gq_ps[p][:], ct[:, j, 0:128], ct[:, j, 128:256],
                                 start=(jj == 0), stop=(jj == N_LOC // 128 - 1))

        def gram_interleave(p):
            """after_plane hook: transpose group g after plane 2g+1, gram MMs
            two groups behind (transpose has a full conv chunk to complete)."""
            gq_ps[p] = ps_gram.tile([128, 128], F32, tag="gq", name=f"gq{p}")

            def hook(t):
                if t % 2 == 1:
                    g = (t - 1) // 2
                    gram_transpose(p, g)
                    if g > 1:
                        gram_mms(p, g - 2)
            return hook

        def gram_pair(p):
            """non-interleaved fallback: full gram for pair p."""
            gq_ps[p] = ps_gram.tile([128, 256], F32, tag="gq", name=f"gq{p}")
            for g in range(NG):
                gram_transpose(p, g)
                gram_mms(p, g)

        def extract_pair(p):
            """S block + diagonals of pair p into arbuf[:, p, :]."""
            nc.vector.tensor_copy(out=arbuf[:, p, 0:128], in_=gq_ps[p][:])
            nc.vector.tensor_reduce(out=arbuf[:, p, 128:129], in_=ssqk[:, p, :],
                                    axis=mybir.AxisListType.X, op=mybir.AluOpType.add)
            nc.vector.tensor_reduce(out=arbuf[:, p, 129:130], in_=ssqk[:, 2 + p, :],
                                    axis=mybir.AxisListType.X, op=mybir.AluOpType.add)

        ar_in = [dram.tile([128, 130], F32, name=f"ar_in{p}") for p in range(2)]
        ar_out = [dram.tile([128, 130], F32, name=f"ar_out{p}") for p in range(2)]
        gar = singles.tile([128, 2, 130], F32)

        def launch_ar(p):
            nc.gpsimd.dma_start(out=ar_in[p][:], in_=arbuf[:, p, :])
            if sim:
                nc.gpsimd.dma_start(out=ar_out[p][:], in_=ar_in[p][:])
            else:
                nc.gpsimd.collective_compute(
                    "AllReduce", mybir.AluOpType.add,
                    replica_groups=[list(range(N_CORES))],
                    ins=[ar_in[p].opt()], outs=[ar_out[p].opt()])
            nc.gpsimd.dma_start(out=gar[:, p, :], in_=ar_out[p][:])

        # ---- per-pair: normalization, softmax, B, compose G ----
        rno = singles.tile([128, 2, 2], F32)
        rqs = singles.tile([128, 2], F32)
        rkb = singles.tile([128, 2, 128], F32)
        rkrow = singles.tile([1, 2, 128], F32)
        bt_sb = [singles.tile([128, DIM], BF16, tag=f"bt{p}", name=f"bt{p}")
                 for p in range(2)]
        # G = G0 + G1 (out = G0@x + G1@x = (G0+G1)@x): pair 0 stages to gv0,
        # pair 1's compose evacuation adds into gsum
        gv0 = singles.tile([128, NVSLOT, DIM], F32)
        gsum = singles.tile([128, NVSLOT, DIM], BF16)

        def chain_pair(p):
            """normalize + softmax + B^T + compose G_p (PE ops: transpose, 2 btp,
            14 compose MMs)."""
            # rno = exp(-0.5*ln(diag)) = rsqrt(diag); same act table as Exp
            nc.scalar.activation(out=rno[:, p, :], in_=gar[:, p, 128:130], func=LN)
            nc.scalar.activation(out=rno[:, p, :], in_=rno[:, p, :], func=EXP,
                                 scale=-0.5)
            nc.vector.tensor_mul(rqs[:, p:p + 1], rno[:, p, 0:1], tsc[:, p:p + 1])
            # rk column -> row (PE transpose) -> all partitions (gpsimd bcast)
            tps = ps_gram.tile([128, 128], F32, tag="aux", name=f"tps{p}")
            nc.tensor.transpose(tps[0:1, :], rno[:, p, 1:2], eye_sb[:])
            nc.vector.tensor_copy(out=rkrow[:, p, :], in_=tps[0:1, :])
            nc.gpsimd.partition_broadcast(rkb[:, p, :], rkrow[:, p, :])

            lg = small_p.tile([128, 128], F32, tag="lg")
            nc.vector.tensor_mul(lg[:], gar[:, p, 0:128], rkb[:, p, :])
            nc.vector.tensor_scalar_mul(lg[:], lg[:], rqs[:, p:p + 1])
            btp = ps_gram.tile([128, 128], F32, tag="aux", name=f"btp{p}")
            mx = small_p.tile([128, 1], F32, tag="mx")
            at = small_p.tile([128, 64], F32, tag="at")
            sm = small_p.tile([128, 1], F32, tag="sm")
            for hf in range(2):
                hs = slice(hf * 64, (hf + 1) * 64)
                sub = lg[hs, hf * 64:(hf + 1) * 64]
                nc.vector.tensor_reduce(out=mx[hs], in_=sub, axis=mybir.AxisListType.X,
                                        op=mybir.AluOpType.max, negate=True)
                nc.scalar.activation(out=at[hs], in_=sub, func=EXP,
                                     bias=mx[hs], scale=1.0)
                nc.vector.tensor_reduce(out=sm[hs], in_=at[hs],
                                        axis=mybir.AxisListType.X,
                                        op=mybir.AluOpType.add)
                nc.vector.reciprocal(out=sm[hs], in_=sm[hs])
                nc.vector.tensor_scalar_mul(at[hs], at[hs], sm[hs])
                # B_h^T = attn_h^T @ projT_h  (partitions hf*64.. aligned throughout)
                nc.tensor.matmul(btp[hs, 0:DIM], at[hs], projt_sb[hs, p, :],
                                 start=True, stop=True)
            nc.vector.tensor_copy(out=bt_sb[p][:], in_=btp[:, 0:DIM])
            # compose G_p: per slot GT[(band,c), e] = MfoldT_v,p . B_p
            for slot in range(NVSLOT):
                gts = ps_gram.tile([128, 128], F32, tag="aux", name=f"gts{p}_{slot}")
                nc.tensor.matmul(gts[:, 0:DIM], dwtv_sb[:, slot, p, :], bt_sb[p][:],
                                 start=True, stop=True)
                if p == 0:
                    nc.vector.tensor_copy(out=gv0[:, slot, :], in_=gts[:, 0:DIM])
                else:
                    nc.vector.tensor_add(gsum[:, slot, :], gts[:, 0:DIM],
                                         gv0[:, slot, :])

        # ---- conv phase with gram + pair-0 chain interleaved ----
        if stop_after == "inputs":
            nc.gpsimd.dma_start(out=out_d[:, 0], in_=xa[0:64, 0, 0:8, 0:64])
            return
        for mi, mac in enumerate(macro_order):
            hook = None
            if stop_after != "convonly":
                if mi == 2:
                    hook = gram_interleave(0)
                elif mi == 3:
                    hook = gram_interleave(1)
            conv_macro(mac, after_plane=hook)
            if mi == 2 and stop_after != "convonly":
                gram_mms(0, NG - 2)
                gram_mms(0, NG - 1)
                extract_pair(0)
                launch_ar(0)
        if stop_after == "convonly":
            return
        gram_mms(1, NG - 2)
        chain_pair(0)
        gram_mms(1, NG - 1)
        extract_pair(1)
        launch_ar(1)

        if stop_after in ("conv",):
            return

        # ---- v-conv: out = G0@x + G1@x, col-tiled 2 t-planes per pass ----
        def vslot_rhs(t, slot):
            if slot < 9:
                # A-slots: taps (dti, dhi, 0) + (dti, dhi, 1)
                dti, dhi = slot // 3, slot % 3
                return xa[:, t + dti, dhi:dhi + 8, 1:65]
            if slot < 12:
                # B-slots: taps (dti, 0, 2) + (dti, 1, 2)
                return xb[:, t + (slot - 9), 0:8, 3:67]
            # C'-slots: tap (dti, 2, 2) via xb band0, band1 weight 0
            return xb[:, t + (slot - 12), 2:10, 3:67]

        def vslot_valid(t, slot):
            dti = slot // 3 if slot < 9 else (slot - 9 if slot < 12 else slot - 12)
            return 0 <= t + dti - 1 <= T - 1

        def vconv():
            """out plane = (G0+G1) @ x, 15 slots per t-plane, M=64;
            two t-planes run concurrently in separate PE column groups."""
            for tp in range(T // 2):
                t0, t1 = 2 * tp, 2 * tp + 1
                fo = ps_fo.tile([128, NT], F32, tag="fo")
                sl0 = [s for s in range(NVSLOT) if vslot_valid(t0, s)]
                sl1 = [s for s in range(NVSLOT) if vslot_valid(t1, s)]
                for s in range(NVSLOT):
                    if s in sl0:
                        nc.tensor.matmul(fo[0:64, :], gsum[:, s, :], vslot_rhs(t0, s),
                                         start=(s == sl0[0]), stop=(s == sl0[-1]),
                                         tile_position=(0, 0))
                    if s in sl1:
                        nc.tensor.matmul(fo[64:128, :], gsum[:, s, :], vslot_rhs(t1, s),
                                         start=(s == sl1[0]), stop=(s == sl1[-1]),
                                         tile_position=(0, 64))
                ot = out_p.tile([128, NT], F32, tag="ot")
                if tp % 2 == 0:
                    nc.vector.tensor_copy(out=ot[:], in_=fo[:])
                else:
                    nc.scalar.copy(out=ot[:], in_=fo[:])
                nc.sync.dma_start(out=out_d[:, t0],
                                  in_=ot[0:64].rearrange("p (h w) -> p h w", h=HL))
                nc.sync.dma_start(out=out_d[:, t1],
                                  in_=ot[64:128].rearrange("p (h w) -> p h w", h=HL))

        chain_pair(1)
        vconv()


def _prep_inputs(x, qkv_w, dw_w, proj_w, temperature):
    """Host-side sharding + weight layout."""
    b, c, t, h, w = x.shape
    w1 = qkv_w.reshape(C3H, DIM).astype(np.float64)   # (768, 64)
    dw = dw_w.reshape(C3H, 4, 3, 3, 3).astype(np.float64)
    # folded conv: M[o, c, dti, dhi, dwi] = sum_j dw[o, j, taps] * w1[4*(o//4)+j, c]
    j_idx = (np.arange(C3H) // 4) * 4
    w1g = w1[j_idx[:, None] + np.arange(4)[None, :], :]      # (768, 4, 64)
    mfold = np.einsum("ojtuv,ojc->octuv", dw, w1g)           # (768, 64, 3,3,3)

    # ---- fp8 q,k weights: [c_band(128), mac(4), slot(9), ko(2), o(128)] ----
    mqk = mfold[0:512]                                       # (512, 64, 3,3,3)
    S = 2.0 ** np.floor(np.log2(120.0 / np.abs(mqk).max()))
    dwt8 = np.zeros((128, 4, 9, 2, 128), dtype=np.float32)
    for mac in range(4):
        osl = slice(mac * 128, (mac + 1) * 128)
        for dt in range(3):
            for dh in range(3):
                for bb in range(2):
                    for s in range(2):
                        dwi = 2 * bb + s
                        if dwi > 2:
                            continue
                        dwt8[64 * bb:64 * bb + 64, mac, 3 * dt + dh, s, :] = \
                            (S * mqk[osl, :, dt, dh, dwi]).T
    dwt8 = np.clip(dwt8, -240, 240).astype(ml_dtypes.float8_e4m3fn)

    # ---- fp8 x image: xq8[c, tt, hh, ww] padded (t+-1, h+-1, w: -1..+2) ----
    xq = np.zeros((c, XT, H + 2, W + 3), dtype=np.float32)
    xq[:, 1:T + 1, 1:H + 1, 1:W + 1] = x[0]
    xq8 = xq.astype(ml_dtypes.float8_e4m3fn)

    # ---- v-conv slots (bf16 path) ----
    slots = []
    for dti in range(3):
        for dhi in range(3):
            slots.append(((dti, dhi, 0), (dti, dhi, 1)))     # A-pairs
    for dti in range(3):
        slots.append(((dti, 0, 2), (dti, 1, 2)))             # B-pairs (h-shift band)
    for dti in range(3):
        slots.append(((dti, 2, 2), None))                    # C'-singles
    # dwtv[s, p, o, 64b + c] = mfold[512 + 128p + o, c, tap(s, b)]
    dwtv = np.zeros((NVSLOT, 2, 128, 128), dtype=np.float32)
    for si, (tap0, tap1) in enumerate(slots):
        for p in range(2):
            osl = slice(512 + p * 128, 512 + (p + 1) * 128)
            dwtv[si, p, :, 0:64] = mfold[osl, :, tap0[0], tap0[1], tap0[2]]
            if tap1 is not None:
                dwtv[si, p, :, 64:128] = mfold[osl, :, tap1[0], tap1[1], tap1[2]]
    dwtv = dwtv.astype(ml_dtypes.bfloat16)
    pw = proj_w.reshape(DIM, HEADS, DIM)              # (e, h, c)
    # projt[hf*64+c, p, e] = proj_w[e, (2p+hf)*64 + c]
    projt = np.zeros((128, 2, DIM), dtype=np.float32)
    for p in range(2):
        for hf in range(2):
            projt[hf * 64:(hf + 1) * 64, p, :] = pw[:, 2 * p + hf, :].T
    temp = np.asarray(temperature, dtype=np.float32).reshape(HEADS)
    eye = np.eye(128, dtype=np.float32)

    xp = np.zeros((c, t, h + 2, w), dtype=np.float32)
    xp[:, :, 1:h + 1, :] = x[0]
    in_maps = []
    for i in range(N_CORES):
        R0 = i * HL
        # fp8 staging [128, 18, 2, 640] (device pads sub-planes to stride 1024)
        xs8 = np.zeros((128, XT, 2, SUBV), dtype=ml_dtypes.float8_e4m3fn)
        for bb in range(2):
            for s in range(2):
                sl = xq8[:, :, R0:R0 + HLH, 2 * bb + s:2 * bb + s + W]
                xs8[64 * bb:64 * bb + 64, :, s, :] = sl.reshape(c, XT, SUBV)
        # bf16 v-conv image
        xsb = np.zeros((c, PT, PH, PW), dtype=np.float32)
        xsb[:, 1:T + 1, :, 2:W + 2] = xp[:, :, R0:R0 + HLH, :]
        xsb = xsb.reshape(c, PT * PH * PW).astype(ml_dtypes.bfloat16)
        in_maps.append({"xs8": xs8, "dwt8": dwt8, "x": xsb, "dwtv": dwtv,
                        "projt": projt, "temp": temp, "eye": eye})
    return in_maps


def kernel(x, qkv_w, dw_w, proj_w, temperature, _trace=False):
    if "nc" not in _CACHE:
        _CACHE["nc"] = _build()
    nc = _CACHE["nc"]
    in_maps = _prep_inputs(np.asarray(x, np.float32), np.asarray(qkv_w, np.float32),
                           np.asarray(dw_w, np.float32), np.asarray(proj_w, np.float32),
                           np.asarray(temperature, np.float32))
    kw = {}
    if _trace:
        kw = dict(trace=True, stitch_traces=True, trace_cores=list(range(N_CORES)))
    res = run_bass_kernel_spmd(nc, in_maps, core_ids=list(range(N_CORES)), **kw)
    _CACHE["last_res"] = res
    out = np.zeros((1, DIM, T, H, W), dtype=np.float32)
    for i in range(N_CORES):
        out[0, :, :, i * HL:(i + 1) * HL, :] = res.results[i]["out"]
    return out
